# revision 1
# baseline (speedup 1.0000x reference)
"""Trainium2 Bass kernel for nn_DiffPoolEncoder (batch of 64 graphs, 64 nodes each).

Strategy: data-parallel over graphs — 8 graphs per NeuronCore. Each core gets its
8 graphs' node features plus block-diagonal adjacency slabs (pairs of graphs packed
into 128-partition tiles) and replicated encoder weights.

Math reformulations (validated against the jax reference in numpy):
  * The level-2 pooling branch (pool1_W1/pool1_W2, S2 softmax, S2^T Z2 bmm) never
    affects the output: softmax rows sum to 1, so mean_s(S2^T Z2) = colsum(Z2)/P2.
  * Propagation A@X+X is fed as Ahat = A+I from the host; transposed activations
    are produced directly on the tensor engine via out = lhsT.T @ rhs with
    lhsT = activation (node-major), rhs = Ahat^T — no explicit transposes anywhere.
  * A1^T = S^T A^T S computed per graph-pair in block-diagonal form; the final
    level-2 propagation folds into v = Ahat1^T 1 (free-axis reduce of Ahat1^T).
"""
import numpy as np

NC_COUNT = 8
B, NPG, D = 64, 64, 256
GPC = B // NC_COUNT     # graphs per core = 8
PAIRS = GPC // 2        # 4 pairs of graphs (128 nodes) per core

_BUILT = None


def _build():
    import concourse.bacc as bacc
    import concourse.tile as tile
    from concourse import mybir
    from concourse.masks import make_identity

    f32 = mybir.dt.float32
    f32r = mybir.dt.float32r
    nc = bacc.Bacc("TRN2", target_bir_lowering=False, debug=False,
                   num_devices=NC_COUNT)

    # ---- DRAM I/O (per-core shapes; weights pre-arranged to SBUF layout) ----
    xc_d = nc.dram_tensor("xc", [PAIRS, 128, 256], f32r, kind="ExternalInput")
    ahatT_d = nc.dram_tensor("ahat_t", [PAIRS, 128, 128], f32r, kind="ExternalInput")
    araw_d = nc.dram_tensor("a_raw", [PAIRS, 128, 128], f32r, kind="ExternalInput")
    w1cat_d = nc.dram_tensor("w1cat", [128, 2, 512], f32r, kind="ExternalInput")
    w2cat_d = nc.dram_tensor("w2cat", [128, 2, 288], f32r, kind="ExternalInput")
    w1e2_d = nc.dram_tensor("w1e2", [128, 2, 256], f32r, kind="ExternalInput")
    w2e2_d = nc.dram_tensor("w2e2", [128, 2, 2, 128], f32, kind="ExternalInput")
    lin1_d = nc.dram_tensor("lin1", [128, 2, 4, 128], f32, kind="ExternalInput")
    lin2_d = nc.dram_tensor("lin2", [128, 4, 2, 128], f32, kind="ExternalInput")
    b1t_d = nc.dram_tensor("b1t", [128, 4], f32, kind="ExternalInput")
    b2t_d = nc.dram_tensor("b2t", [128, 2], f32, kind="ExternalInput")
    out_d = nc.dram_tensor("out_t", [2, 128, 8], f32, kind="ExternalOutput")

    with tile.TileContext(nc) as tc:
        with (
            tc.tile_pool(name="singles", bufs=1) as singles,
            tc.tile_pool(name="work", bufs=3) as work,
            tc.tile_pool(name="small", bufs=4) as small,
            tc.tile_pool(name="ps_t1t", bufs=2, space="PSUM") as ps_t1t,
            tc.tile_pool(name="ps_h", bufs=2, space="PSUM") as ps_h,
            tc.tile_pool(name="ps_t2t", bufs=2, space="PSUM") as ps_t2t,
            tc.tile_pool(name="ps_misc", bufs=2, space="PSUM") as ps_misc,
        ):
            Relu = mybir.ActivationFunctionType.Relu

            def copy_op(i, out, in_):
                if i % 2 == 0:
                    nc.vector.tensor_copy(out=out, in_=in_)
                else:
                    nc.scalar.copy(out=out, in_=in_)

            def relu_op(i, out, in_):
                if i % 2 == 0:
                    nc.scalar.activation(out=out, in_=in_, func=Relu)
                else:
                    nc.vector.tensor_scalar_max(out=out, in0=in_, scalar1=0.0)

            # ---- DMA loads, ordered by first use ----
            ident = singles.tile([128, 128], f32)
            make_identity(nc, ident)

            X_sb = [None] * PAIRS
            ahatT_sb = [None] * PAIRS
            araw_sb = [None] * PAIRS

            def load_pair(p):
                X_sb[p] = singles.tile([128, 256], f32r, tag=f"x{p}", name=f"x{p}")
                nc.sync.dma_start(out=X_sb[p], in_=xc_d[p])
                ahatT_sb[p] = singles.tile([128, 128], f32r, tag=f"ahatT{p}",
                                           name=f"ahatT{p}")
                nc.sync.dma_start(out=ahatT_sb[p], in_=ahatT_d[p])

            load_pair(0)
            w1cat = singles.tile([128, 2, 512], f32r)
            nc.sync.dma_start(out=w1cat, in_=w1cat_d[:, :, :])
            load_pair(1)
            w2cat = singles.tile([128, 2, 288], f32r)
            nc.sync.dma_start(out=w2cat, in_=w2cat_d[:, :, :])
            load_pair(2)
            load_pair(3)
            for p in range(PAIRS):
                araw_sb[p] = singles.tile([128, 128], f32r, tag=f"araw{p}",
                                          name=f"araw{p}")
                nc.sync.dma_start(out=araw_sb[p], in_=araw_d[p])
            w1e2 = singles.tile([128, 2, 256], f32r)
            nc.sync.dma_start(out=w1e2, in_=w1e2_d[:, :, :])
            w2e2 = singles.tile([128, 2, 2, 128], f32)
            nc.sync.dma_start(out=w2e2, in_=w2e2_d[:, :, :, :])
            lin1 = singles.tile([128, 2, 4, 128], f32)
            nc.sync.dma_start(out=lin1, in_=lin1_d[:, :, :, :])
            lin2 = singles.tile([128, 4, 2, 128], f32)
            nc.sync.dma_start(out=lin2, in_=lin2_d[:, :, :, :])
            b1t = singles.tile([128, 4], f32)
            nc.sync.dma_start(out=b1t, in_=b1t_d[:, :])
            b2t = singles.tile([128, 2], f32)
            nc.sync.dma_start(out=b2t, in_=b2t_d[:, :])

            # ---- persistent level-1 outputs ----
            X1_sb = [singles.tile([128, 256], f32r, tag=f"X1_{j}", name=f"X1_{j}")
                     for j in range(2)]
            a1t_sb = [singles.tile([128, 128], f32r, tag=f"a1t_{j}", name=f"a1t_{j}")
                      for j in range(2)]
            for j in range(2):
                nc.gpsimd.memset(a1t_sb[j].bitcast(f32), 0.0)
            Ut = [[singles.tile([128, 4], f32, tag=f"Ut_{c}_{j}", name=f"Ut_{c}_{j}")
                   for j in range(2)] for c in range(2)]

            # ====== level 1: stage-sliced over pair groups of two ======
            for g in range(2):
                prs = [2 * g, 2 * g + 1]
                t1t_ps, t1t, h_ps, h_sb, t2t_ps, t2t = {}, {}, {}, {}, {}, {}
                sz_ps, s_bd, z_sb, pl_ps = {}, {}, {}, {}
                # T1t = X^T AhatT  (per pair, 2 chunk mms)
                for p in prs:
                    t1t_ps[p] = ps_t1t.tile([128, 2, 128], f32, tag="t1t",
                                            name="t1t_ps")
                    for c in range(2):
                        nc.tensor.matmul(t1t_ps[p][:, c, :],
                                         X_sb[p][:, 128 * c:128 * (c + 1)],
                                         ahatT_sb[p], start=True, stop=True)
                for i, p in enumerate(prs):
                    t1t[p] = work.tile([128, 2, 128], f32r, tag="t1t_sb", name="t1t")
                    copy_op(i, t1t[p], t1t_ps[p])
                # H = relu(T1 @ [W1p | W1e])
                for p in prs:
                    h_ps[p] = ps_h.tile([128, 512], f32, tag="h", name="h_ps")
                    for c in range(2):
                        nc.tensor.matmul(h_ps[p], t1t[p][:, c, :],
                                         w1cat[:, c, :],
                                         start=(c == 0), stop=(c == 1))
                for i, p in enumerate(prs):
                    h_sb[p] = work.tile([128, 512], f32r, tag="h_sb", name="h_sb")
                    relu_op(i, h_sb[p], h_ps[p])
                # T2t = (Ahat @ H)^T  (4 chunk mms into one bank)
                for p in prs:
                    t2t_ps[p] = ps_t2t.tile([128, 4, 128], f32, tag="t2t",
                                            name="t2t_ps")
                    for c in range(4):
                        nc.tensor.matmul(t2t_ps[p][:, c, :],
                                         h_sb[p][:, 128 * c:128 * (c + 1)],
                                         ahatT_sb[p], start=True, stop=True)
                for i, p in enumerate(prs):
                    t2t[p] = work.tile([128, 4, 128], f32r, tag="t2t_sb", name="t2t")
                    copy_op(i + 1, t2t[p], t2t_ps[p])
                # [S_logits | Z]
                for p in prs:
                    sz_ps[p] = ps_misc.tile([128, 288], f32, tag="misc", name="sz_ps")
                    for c in range(2):
                        nc.tensor.matmul(sz_ps[p][:, 0:32], t2t[p][:, c, :],
                                         w2cat[:, c, 0:32],
                                         start=(c == 0), stop=(c == 1))
                    for c in range(2):
                        nc.tensor.matmul(sz_ps[p][:, 32:288],
                                         t2t[p][:, 2 + c, :],
                                         w2cat[:, c, 32:288],
                                         start=(c == 0), stop=(c == 1))
                # softmax + Z copy
                for i, p in enumerate(prs):
                    negmax = small.tile([128, 1], f32, tag="negmax", name="negmax")
                    nc.vector.reduce_max(out=negmax, in_=sz_ps[p][:, 0:32],
                                         axis=mybir.AxisListType.X, negate=True)
                    s_exp = small.tile([128, 32], f32, tag="s_exp", name="s_exp")
                    sumexp = small.tile([128, 1], f32, tag="sumexp", name="sumexp")
                    nc.scalar.activation(out=s_exp, in_=sz_ps[p][:, 0:32],
                                         func=mybir.ActivationFunctionType.Exp,
                                         bias=negmax, scale=1.0, accum_out=sumexp)
                    rsum = small.tile([128, 1], f32, tag="rsum", name="rsum")
                    nc.vector.reciprocal(out=rsum, in_=sumexp)
                    s_bd[p] = work.tile([128, 64], f32r, tag="s_bd", name="s_bd")
                    nc.gpsimd.memset(s_bd[p].bitcast(f32), 0.0)
                    nc.scalar.mul(out=s_bd[p][0:64, 0:32], in_=s_exp[0:64, :],
                                  mul=rsum[0:64, :])
                    nc.vector.tensor_scalar_mul(out=s_bd[p][64:128, 32:64],
                                                in0=s_exp[64:128, :],
                                                scalar1=rsum[64:128, :])
                    z_sb[p] = work.tile([128, 256], f32r, tag="z_sb", name="z_sb")
                    copy_op(i, z_sb[p], sz_ps[p][:, 32:288])
                # pooling: X1 rows, M2, A1t  (one combined psum bank per pair)
                for p in prs:
                    pl_ps[p] = ps_misc.tile([128, 384], f32, tag="misc", name="pl_ps")
                    nc.tensor.matmul(pl_ps[p][0:64, 0:256], s_bd[p], z_sb[p],
                                     start=True, stop=True)
                    nc.tensor.matmul(pl_ps[p][:, 256:320], araw_sb[p], s_bd[p],
                                     start=True, stop=True)
                for i, p in enumerate(prs):
                    j, half = p // 2, 64 * (p % 2)
                    nc.vector.tensor_copy(out=X1_sb[j][half:half + 64, :],
                                          in_=pl_ps[p][0:64, 0:256])
                    m2_sb = small.tile([128, 64], f32r, tag="m2_sb", name="m2_sb")
                    nc.scalar.copy(out=m2_sb, in_=pl_ps[p][:, 256:320])
                    nc.tensor.matmul(pl_ps[p][0:64, 320:384], s_bd[p], m2_sb,
                                     start=True, stop=True)
                    nc.vector.tensor_add(
                        out=a1t_sb[j][half:half + 64, half:half + 64],
                        in0=pl_ps[p][0:64, 320:384], in1=ident[0:64, 0:64])

            # ====== level 2: stage-sliced over the two chunks ======
            t1l2_ps, t1l2, h2_ps, h2, v_bd, u_ps = {}, {}, {}, {}, {}, {}
            for j in range(2):
                t1l2_ps[j] = ps_t1t.tile([128, 2, 128], f32, tag="t1t",
                                         name="t1l2_ps")
                for c in range(2):
                    nc.tensor.matmul(t1l2_ps[j][:, c, :],
                                     X1_sb[j][:, 128 * c:128 * (c + 1)],
                                     a1t_sb[j], start=True, stop=True)
            for j in range(2):
                t1l2[j] = work.tile([128, 2, 128], f32r, tag="t1t_sb", name="t1l2")
                copy_op(j, t1l2[j], t1l2_ps[j])
                v_bd[j] = small.tile([128, 4], f32r, tag="v_bd", name="v_bd")
                nc.gpsimd.memset(v_bd[j].bitcast(f32), 0.0)
                vsum = small.tile([128, 1], f32, tag="vsum", name="vsum")
                nc.vector.reduce_sum(out=vsum, in_=a1t_sb[j],
                                     axis=mybir.AxisListType.X)
                for gg in range(4):
                    nc.vector.tensor_copy(out=v_bd[j][32 * gg:32 * (gg + 1),
                                                      gg:gg + 1],
                                          in_=vsum[32 * gg:32 * (gg + 1), :])
            for j in range(2):
                h2_ps[j] = ps_h.tile([128, 256], f32, tag="h", name="h2_ps")
                for c in range(2):
                    nc.tensor.matmul(h2_ps[j], t1l2[j][:, c, :],
                                     w1e2[:, c, :],
                                     start=(c == 0), stop=(c == 1))
            for j in range(2):
                h2[j] = work.tile([128, 256], f32r, tag="h2_sb", name="h2")
                relu_op(j, h2[j], h2_ps[j])
            for j in range(2):
                u_ps[j] = ps_misc.tile([128, 2, 4], f32, tag="misc", name="u_ps")
                for c in range(2):
                    nc.tensor.matmul(u_ps[j][:, c, :],
                                     h2[j][:, 128 * c:128 * (c + 1)], v_bd[j],
                                     start=True, stop=True)
            for j in range(2):
                for c in range(2):
                    nc.scalar.mul(out=Ut[c][j], in_=u_ps[j][:, c, :], mul=0.125)

            # ================= tail: Xg_t -> MLP -> output =================
            xgt = [singles.tile([128, 8], f32, tag=f"xgt{m}", name=f"xgt{m}")
                   for m in range(2)]
            for m in range(2):
                xg_ps = ps_misc.tile([128, 8], f32, tag="misc", name="xg_ps")
                for j in range(2):
                    for c in range(2):
                        nc.tensor.matmul(xg_ps[:, 4 * j:4 * (j + 1)],
                                         w2e2[:, c, m, :], Ut[c][j],
                                         start=(c == 0), stop=(c == 1))
                copy_op(m, xgt[m], xg_ps)

            yt = [singles.tile([128, 8], f32, tag=f"yt{m}", name=f"yt{m}")
                  for m in range(4)]
            for m in range(4):
                y_ps = ps_misc.tile([128, 8], f32, tag="misc", name="y_ps")
                for c in range(2):
                    nc.tensor.matmul(y_ps, lin1[:, c, m, :], xgt[c],
                                     start=(c == 0), stop=(c == 1))
                nc.scalar.activation(out=yt[m], in_=y_ps, func=Relu,
                                     bias=b1t[:, m:m + 1], scale=1.0)

            for m in range(2):
                o_ps = ps_misc.tile([128, 8], f32, tag="misc", name="o_ps")
                for c in range(4):
                    nc.tensor.matmul(o_ps, lin2[:, c, m, :], yt[c],
                                     start=(c == 0), stop=(c == 3))
                o_sb = small.tile([128, 8], f32, tag="o_sb", name="o_sb")
                nc.scalar.activation(out=o_sb, in_=o_ps,
                                     func=mybir.ActivationFunctionType.Identity,
                                     bias=b2t[:, m:m + 1], scale=1.0)
                nc.sync.dma_start(out=out_d[m], in_=o_sb)

    nc.finalize()  # Bacc.compile(): wait legalization + register allocation
    return nc


def _prep_in_maps(inputs):
    f32 = np.float32
    x = np.ascontiguousarray(np.asarray(inputs["x"], f32))
    ei = np.asarray(inputs["edge_index"])
    src, dst = np.asarray(ei[0]), np.asarray(ei[1])

    A_blocks = np.zeros((B, NPG, NPG), f32)
    ok = (src // NPG) == (dst // NPG)
    A_blocks[src[ok] // NPG, src[ok] % NPG, dst[ok] % NPG] = 1.0
    I64 = np.eye(NPG, dtype=f32)

    W1p = np.asarray(inputs["pool0_W1"], f32)
    W1e = np.asarray(inputs["emb0_W1"], f32)
    W2p = np.asarray(inputs["pool0_W2"], f32)
    W2e = np.asarray(inputs["emb0_W2"], f32)

    def kmaj(w, nchunk):  # (nchunk*128, n) -> (128, nchunk, n)
        return w.reshape(nchunk, 128, -1).transpose(1, 0, 2)

    shared = {
        "w1cat": np.ascontiguousarray(
            np.concatenate([kmaj(W1p, 2), kmaj(W1e, 2)], axis=2)),          # (128,2,512)
        "w2cat": np.ascontiguousarray(
            np.concatenate([kmaj(W2p, 2), kmaj(W2e, 2)], axis=2)),          # (128,2,288)
        "w1e2": np.ascontiguousarray(kmaj(np.asarray(inputs["emb1_W1"], f32), 2)),
        "w2e2": np.ascontiguousarray(
            np.asarray(inputs["emb1_W2"], f32).reshape(2, 128, 2, 128).transpose(1, 0, 2, 3)),
        "lin1": np.ascontiguousarray(
            np.asarray(inputs["lin1_W"], f32).reshape(2, 128, 4, 128).transpose(1, 0, 2, 3)),
        "lin2": np.ascontiguousarray(
            np.asarray(inputs["lin2_W"], f32).reshape(4, 128, 2, 128).transpose(1, 0, 2, 3)),
        "b1t": np.ascontiguousarray(np.asarray(inputs["lin1_b"], f32).reshape(4, 128).T),
        "b2t": np.ascontiguousarray(np.asarray(inputs["lin2_b"], f32).reshape(2, 128).T),
    }

    in_maps = []
    for c in range(NC_COUNT):
        xc = np.ascontiguousarray(x[c * GPC * NPG:(c + 1) * GPC * NPG].reshape(PAIRS, 128, 256))
        ahatT = np.zeros((PAIRS, 128, 128), f32)
        araw = np.zeros((PAIRS, 128, 128), f32)
        for p in range(PAIRS):
            g0, g1 = GPC * c + 2 * p, GPC * c + 2 * p + 1
            ahatT[p, :64, :64] = (A_blocks[g0] + I64).T
            ahatT[p, 64:, 64:] = (A_blocks[g1] + I64).T
            araw[p, :64, :64] = A_blocks[g0]
            araw[p, 64:, 64:] = A_blocks[g1]
        in_maps.append({"xc": xc, "ahat_t": ahatT, "a_raw": araw, **shared})
    return in_maps


def kernel(**inputs) -> np.ndarray:
    global _BUILT
    from concourse.bass_utils import run_bass_kernel_spmd

    if _BUILT is None:
        _BUILT = _build()
    nc = _BUILT
    in_maps = _prep_in_maps(inputs)
    res = run_bass_kernel_spmd(nc, in_maps, core_ids=list(range(NC_COUNT)))
    out = np.zeros((B, 256), np.float32)
    for c in range(NC_COUNT):
        ot = res.results[c]["out_t"]  # (2, 128, 8)
        out[GPC * c:GPC * (c + 1)] = ot.transpose(2, 0, 1).reshape(GPC, 256)
    return out

